# revision 11
# baseline (speedup 1.0000x reference)
"""Multi-head attention (B=2, S=2048, D=1024, H=16) on 8 TRN2 NeuronCores.

Sharding: tensor-parallel on heads (2 heads = 128 channels per core).
Everything on-device runs in "transposed" layout [channel, B*S]:
  - host passes hiddenT [D, B*S] (bf16) replicated to all cores
  - per-core Q/K/V projections produce qT/kT/vT [128, B*S]
  - attention per (batch, head) in scoresT layout [key, query]:
      scoresT tile = kT_h.T-contraction matmul, exp via ScalarE with
      mask as per-partition bias and 1/sqrt(hd) as scale, softmax
      denominator via an all-ones row appended to V (row 64 of the
      PV accumulator), normalization by DMA-broadcast reciprocal.
  - normalized ctxT (bf16) is AllGathered across cores; each core then
    computes a 128-row slice of outT = Wo @ ctx.T and returns it.
Host concatenates the 8 slices and transposes back to [B, S, D].
"""

import numpy as np
import ml_dtypes

import concourse.bass as bass
import concourse.mybir as mybir
import concourse.tile as tile
from concourse import bacc
from concourse import bass_utils
from concourse.masks import make_identity

F32 = mybir.dt.float32
BF16 = mybir.dt.bfloat16
BF16_NP = ml_dtypes.bfloat16

B, S, D, H = 2, 2048, 1024, 16
HD = D // H
BS = B * S            # 4096
P = 128               # partitions / channels per core
NCORES = 8
KT = S // P           # 16 key tiles per batch
NQ = 512              # matmul moving free dim
VA_W = HD + 1         # v_aug columns per key tile (64 v cols + ones col)

_CACHE = {}


def _build():
    nc = bacc.Bacc("TRN2", target_bir_lowering=False, debug=False,
                   num_devices=NCORES)

    hT = nc.dram_tensor("hT", [D, BS], BF16, kind="ExternalInput")
    wq = nc.dram_tensor("wq", [D, P], BF16, kind="ExternalInput")
    wk = nc.dram_tensor("wk", [D, P], BF16, kind="ExternalInput")
    wv = nc.dram_tensor("wv", [D, P], BF16, kind="ExternalInput")
    wo = nc.dram_tensor("wo", [D, P], BF16, kind="ExternalInput")
    bq = nc.dram_tensor("bq", [P, 1], F32, kind="ExternalInput")
    bk = nc.dram_tensor("bk", [P, 1], F32, kind="ExternalInput")
    bv = nc.dram_tensor("bv", [P, 1], F32, kind="ExternalInput")
    bo = nc.dram_tensor("bo", [P, 1], F32, kind="ExternalInput")
    maskT = nc.dram_tensor("maskT", [S, B], F32, kind="ExternalInput")
    outT = nc.dram_tensor("outT", [P, BS], F32, kind="ExternalOutput")

    with tile.TileContext(nc) as tc:
        with (
            tc.tile_pool(name="const", bufs=1) as const,
            tc.tile_pool(name="res", bufs=1) as res,
            tc.tile_pool(name="ht", bufs=16) as ht_pool,
            tc.tile_pool(name="va", bufs=2) as va_pool,
            tc.tile_pool(name="pr", bufs=3) as pr_pool,
            tc.tile_pool(name="bc", bufs=2) as bc_pool,
            tc.tile_pool(name="g", bufs=8) as g_pool,
            tc.tile_pool(name="ot", bufs=3) as ot_pool,
            tc.tile_pool(name="dram", bufs=1, space="DRAM") as dram,
        ):
            # ---- constants / weights in SBUF ----
            # w*_sb[:, jt*128:(jt+1)*128][p, m] = w*[jt*128 + p, m]
            w_sbs = {}
            for nm, w in (("wq", wq), ("wk", wk), ("wv", wv), ("wo", wo)):
                t = const.tile([P, D], BF16, name=f"{nm}_sb", tag=f"{nm}_sb")
                nc.sync.dma_start(
                    t[:].rearrange("p (j m) -> p j m", j=D // P),
                    w.ap().rearrange("(j p) m -> p j m", p=P))
                w_sbs[nm] = t
            b_sbs = {}
            for nm, bt in (("bq", bq), ("bk", bk), ("bv", bv), ("bo", bo)):
                t = const.tile([P, 1], F32, name=f"{nm}_sb", tag=f"{nm}_sb")
                nc.sync.dma_start(t[:], bt.ap())
                b_sbs[nm] = t
            # mask_sb[:, b*KT + t] = maskT[t*128:(t+1)*128, b]
            mask_sb = const.tile([P, B * KT], F32)
            nc.sync.dma_start(
                mask_sb[:].rearrange("p (b t) -> p b t", b=B),
                maskT.ap().rearrange("(t p) b -> p b t", p=P))
            # identity replicated at base partitions 0 and 64 so transposes of
            # head-1 slices (base partition 64) have a matching-base rhs
            ident = const.tile([P, HD], BF16)
            make_identity(nc, ident[0:HD, :])
            nc.sync.dma_start(ident[HD:P, :], ident[0:HD, :])

            qT = res.tile([P, BS], BF16)
            kT = res.tile([P, BS], BF16)
            vT = res.tile([P, BS], BF16)
            ctxraw = res.tile([P, BS], F32)
            ctxn = res.tile([P, BS], BF16)
            r_sb = res.tile([64, BS], F32)  # rows h*32 (quadrant-aligned)

            # ---- phase A: Q/K/V projections (transposed layout) ----
            with tc.tile_pool(name="proj_ps", bufs=4,
                              space="PSUM") as proj_ps:
                for n in range(BS // NQ):
                    nsl = bass.ts(n, NQ)
                    hts = []
                    for k in range(D // P):
                        htt = ht_pool.tile([P, NQ], BF16, name=f"ht{k}",
                                           tag="ht")
                        nc.sync.dma_start(htt[:], hT.ap()[bass.ts(k, P), nsl])
                        hts.append(htt)
                    for wn, bn, dest in (("wq", "bq", qT), ("wk", "bk", kT),
                                         ("wv", "bv", vT)):
                        ps = proj_ps.tile([P, NQ], F32, name=f"ps_{wn}",
                                          tag="proj")
                        for k in range(D // P):
                            nc.tensor.matmul(
                                ps[:], w_sbs[wn][:, bass.ts(k, P)], hts[k][:],
                                start=(k == 0), stop=(k == D // P - 1))
                        nc.vector.tensor_scalar_add(
                            dest[:, nsl], ps[:], b_sbs[bn][:])

            # ---- phases B (attention) + C (gather & output projection) ----
            with (
                tc.tile_pool(name="sc_ps", bufs=2, space="PSUM") as sc_ps,
                tc.tile_pool(name="ctx_ps", bufs=1, space="PSUM") as ctx_ps,
                tc.tile_pool(name="tp_ps", bufs=1, space="PSUM") as tp_ps,
                tc.tile_pool(name="o_ps", bufs=1, space="PSUM") as o_ps,
            ):
                for b in range(B):
                    boff = b * S
                    for h in range(2):
                        hsl = slice(h * HD, (h + 1) * HD)
                        # v_aug[:, kt*65 : kt*65+64] = v rows for key tile kt,
                        # col kt*65+64 = ones (softmax denominator trick)
                        va = va_pool.tile([P, KT * VA_W], BF16, name="va",
                                          tag="va")
                        nc.vector.memset(va[:], 1.0)
                        for kt in range(KT):
                            tp = tp_ps.tile([P, HD], BF16, name="tp", tag="tp")
                            nc.tensor.transpose(
                                tp[:], vT[hsl, boff + kt * P:boff + (kt + 1) * P],
                                ident[hsl, :])
                            nc.vector.tensor_copy(
                                va[:, kt * VA_W:kt * VA_W + HD], tp[:])
                        for c in range(2):   # 1024-wide query chunks
                            coff = boff + c * 1024
                            ctx = ctx_ps.tile([HD + 1, 1024], F32, name="ctx",
                                              tag="ctx")
                            for kt in range(KT):
                                sct = sc_ps.tile([P, 1024], F32, name="sct",
                                                 tag="sct")
                                ssl = sct[:]
                                for i in range(2):
                                    nc.tensor.matmul(
                                        ssl[:, bass.ts(i, NQ)],
                                        kT[hsl, boff + kt * P:boff + (kt + 1) * P],
                                        qT[hsl, coff + i * NQ:coff + (i + 1) * NQ],
                                        start=True, stop=True)
                                pr = pr_pool.tile([P, 1024], BF16, name="pr",
                                                  tag="pr")
                                nc.scalar.activation(
                                    pr[:], ssl,
                                    mybir.ActivationFunctionType.Exp,
                                    bias=mask_sb[:, b * KT + kt:b * KT + kt + 1],
                                    scale=0.125)
                                for i in range(2):
                                    nc.tensor.matmul(
                                        ctx[:, bass.ts(i, NQ)],
                                        va[:, kt * VA_W:(kt + 1) * VA_W],
                                        pr[:, bass.ts(i, NQ)],
                                        start=(kt == 0), stop=(kt == KT - 1))
                            nc.vector.reciprocal(
                                r_sb[h * 32:h * 32 + 1, coff:coff + 1024],
                                ctx[HD:HD + 1, :])
                            nc.vector.tensor_copy(
                                ctxraw[hsl, coff:coff + 1024], ctx[0:HD, :])

                    # normalize ctxT for this batch and kick off the gather
                    bc = bc_pool.tile([P, S], F32, name="bc", tag="bc")
                    for h in range(2):
                        rb = dram.tile([1, S], F32, name=f"rb{b}{h}",
                                       tag=f"rb{b}{h}")
                        nc.sync.dma_start(rb[:], r_sb[h * 32:h * 32 + 1, boff:boff + S])
                        nc.sync.dma_start(
                            bc[h * HD:(h + 1) * HD, :],
                            rb[:].to_broadcast((HD, S)))
                    nc.vector.tensor_mul(
                        ctxn[:, boff:boff + S], ctxraw[:, boff:boff + S], bc[:])

                    cc_in = dram.tile([P, S], BF16, name=f"cc_in{b}",
                                      tag=f"cc_in{b}")
                    cc_out = dram.tile([NCORES * P, S], BF16,
                                       name=f"cc_out{b}", tag=f"cc_out{b}")
                    nc.sync.dma_start(cc_in[:], ctxn[:, boff:boff + S])
                    nc.gpsimd.collective_compute(
                        "AllGather",
                        mybir.AluOpType.bypass,
                        replica_groups=[list(range(NCORES))],
                        ins=[cc_in[:].opt()],
                        outs=[cc_out[:].opt()],
                    )

                    # output projection for this batch's 2048 columns
                    for n in range(S // NQ):
                        po = o_ps.tile([P, NQ], F32, name="po", tag="po")
                        for j in range(D // P):
                            gt = g_pool.tile([P, NQ], BF16, name="gt", tag="gt")
                            nc.sync.dma_start(
                                gt[:], cc_out[bass.ts(j, P), bass.ts(n, NQ)])
                            nc.tensor.matmul(
                                po[:], w_sbs["wo"][:, bass.ts(j, P)], gt[:],
                                start=(j == 0), stop=(j == D // P - 1))
                        ot = ot_pool.tile([P, NQ], F32, name="ot", tag="ot")
                        nc.vector.tensor_scalar_add(ot[:], po[:],
                                                    b_sbs["bo"][:])
                        nc.sync.dma_start(
                            outT.ap()[:, boff + n * NQ:boff + (n + 1) * NQ],
                            ot[:])

    nc.compile()
    return nc


def _prep_inputs(hidden_state, attention_mask, Wq, bq, Wk, bk, Wv, bv, Wo, bo):
    h2 = np.ascontiguousarray(
        np.asarray(hidden_state, dtype=np.float32).reshape(BS, D).T
    ).astype(BF16_NP)
    maskT = np.ascontiguousarray(
        np.asarray(attention_mask, dtype=np.float32).reshape(B, S).T)
    in_maps = []
    for c in range(NCORES):
        sl = slice(c * P, (c + 1) * P)
        in_maps.append({
            "hT": h2,
            "wq": np.ascontiguousarray(np.asarray(Wq)[sl, :].T).astype(BF16_NP),
            "wk": np.ascontiguousarray(np.asarray(Wk)[sl, :].T).astype(BF16_NP),
            "wv": np.ascontiguousarray(np.asarray(Wv)[sl, :].T).astype(BF16_NP),
            "wo": np.ascontiguousarray(np.asarray(Wo)[sl, :].T).astype(BF16_NP),
            "bq": np.asarray(bq, dtype=np.float32)[sl].reshape(P, 1),
            "bk": np.asarray(bk, dtype=np.float32)[sl].reshape(P, 1),
            "bv": np.asarray(bv, dtype=np.float32)[sl].reshape(P, 1),
            "bo": np.asarray(bo, dtype=np.float32)[sl].reshape(P, 1),
            "maskT": maskT,
        })
    return in_maps


def kernel(**inputs) -> np.ndarray:
    if "nc" not in _CACHE:
        _CACHE["nc"] = _build()
    nc = _CACHE["nc"]
    in_maps = _prep_inputs(**inputs)
    res = bass_utils.run_bass_kernel_spmd(
        nc, in_maps, core_ids=list(range(NCORES)))
    outT = np.concatenate([res.results[c]["outT"] for c in range(NCORES)],
                          axis=0)            # [D, BS]
    return np.ascontiguousarray(outT.T).reshape(B, S, D).astype(np.float32)


# revision 12
# speedup vs baseline: 1.1310x; 1.1310x over previous
"""Multi-head attention (B=2, S=2048, D=1024, H=16) on 8 TRN2 NeuronCores.

Sharding: tensor-parallel on heads (2 heads = 128 channels per core).
Everything on-device runs in "transposed" layout [channel, B*S]:
  - host passes hiddenT [D, B*S] (bf16) replicated to all cores
  - per-core Q/K/V projections produce qT/kT/vT [128, B*S]
  - attention per (batch, head) in scoresT layout [key, query]:
      scoresT tile via matmul contracting the head dim, exp on ScalarE
      with the mask as per-partition bias and 1/sqrt(hd) as scale, the
      softmax denominator via an all-ones row appended to V (row 64 of
      the PV accumulator), normalization by DMA-broadcast reciprocal.
  - normalized ctxT (bf16) is AllGathered across cores in 512-column
    chunks; each core then computes a 128-row slice of outT = Wo @
    ctx.T per chunk and returns it.
Host concatenates the 8 slices and transposes back to [B, S, D].

Phase emission order is chosen so the TensorE always has ready "filler"
matmuls (projections for the other batch, output projection for the
previous batch) during the ScalarE-bound attention inner loop — this
both hides those phases and keeps the PE HAM clock-gate warm.
"""

import numpy as np
import ml_dtypes

import concourse.bass as bass
import concourse.mybir as mybir
import concourse.tile as tile
from concourse import bacc
from concourse import bass_utils
from concourse.masks import make_identity

F32 = mybir.dt.float32
BF16 = mybir.dt.bfloat16
BF16_NP = ml_dtypes.bfloat16

B, S, D, H = 2, 2048, 1024, 16
HD = D // H
BS = B * S            # 4096
P = 128               # partitions / channels per core
NCORES = 8
KT = S // P           # 16 key tiles per batch
NQ = 512              # matmul moving free dim
VA_W = HD + 1         # v_aug columns per key tile (64 v cols + ones col)
GC = 512              # gather / output-projection chunk (columns)

_CACHE = {}


def _build():
    nc = bacc.Bacc("TRN2", target_bir_lowering=False, debug=False,
                   num_devices=NCORES)

    hT = nc.dram_tensor("hT", [D, BS], BF16, kind="ExternalInput")
    wq = nc.dram_tensor("wq", [D, P], BF16, kind="ExternalInput")
    wk = nc.dram_tensor("wk", [D, P], BF16, kind="ExternalInput")
    wv = nc.dram_tensor("wv", [D, P], BF16, kind="ExternalInput")
    wo = nc.dram_tensor("wo", [D, P], BF16, kind="ExternalInput")
    bq = nc.dram_tensor("bq", [P, 1], F32, kind="ExternalInput")
    bk = nc.dram_tensor("bk", [P, 1], F32, kind="ExternalInput")
    bv = nc.dram_tensor("bv", [P, 1], F32, kind="ExternalInput")
    bo = nc.dram_tensor("bo", [P, 1], F32, kind="ExternalInput")
    maskT = nc.dram_tensor("maskT", [S, B], F32, kind="ExternalInput")
    outT = nc.dram_tensor("outT", [P, BS], F32, kind="ExternalOutput")

    with tile.TileContext(nc) as tc:
        with (
            tc.tile_pool(name="const", bufs=1) as const,
            tc.tile_pool(name="res", bufs=1) as res,
            tc.tile_pool(name="ht", bufs=16) as ht_pool,
            tc.tile_pool(name="va", bufs=2) as va_pool,
            tc.tile_pool(name="pr", bufs=3) as pr_pool,
            tc.tile_pool(name="bc", bufs=2) as bc_pool,
            tc.tile_pool(name="g", bufs=8) as g_pool,
            tc.tile_pool(name="ot", bufs=3) as ot_pool,
            tc.tile_pool(name="dram", bufs=1, space="DRAM") as dram,
            # PSUM: acc 2x[128,512](2 banks) + sc 2x[128,1024](4) +
            #       ctx 1x[65,1024](2) = 8 banks
            tc.tile_pool(name="acc_ps", bufs=2, space="PSUM") as acc_ps,
            tc.tile_pool(name="sc_ps", bufs=2, space="PSUM") as sc_ps,
            tc.tile_pool(name="ctx_ps", bufs=1, space="PSUM") as ctx_ps,
        ):
            # ---- constants / weights in SBUF ----
            w_sbs = {}
            for nm, w in (("wq", wq), ("wk", wk), ("wv", wv), ("wo", wo)):
                t = const.tile([P, D], BF16, name=f"{nm}_sb", tag=f"{nm}_sb")
                nc.sync.dma_start(
                    t[:].rearrange("p (j m) -> p j m", j=D // P),
                    w.ap().rearrange("(j p) m -> p j m", p=P))
                w_sbs[nm] = t
            b_sbs = {}
            for nm, bt in (("bq", bq), ("bk", bk), ("bv", bv), ("bo", bo)):
                t = const.tile([P, 1], F32, name=f"{nm}_sb", tag=f"{nm}_sb")
                nc.sync.dma_start(t[:], bt.ap())
                b_sbs[nm] = t
            mask_sb = const.tile([P, B * KT], F32)
            nc.sync.dma_start(
                mask_sb[:].rearrange("p (b t) -> p b t", b=B),
                maskT.ap().rearrange("(t p) b -> p b t", p=P))
            # identity replicated at base partitions 0 and 64 so transposes
            # of head-1 slices (base partition 64) have a matching-base rhs
            ident = const.tile([P, HD], BF16)
            make_identity(nc, ident[0:HD, :])
            nc.sync.dma_start(ident[HD:P, :], ident[0:HD, :])

            qT = res.tile([P, BS], BF16)
            kT = res.tile([P, BS], BF16)
            vT = res.tile([P, BS], BF16)
            ctxraw = res.tile([P, BS], F32)
            ctxn = res.tile([P, BS], BF16)
            s_sb = res.tile([64, BS], F32)   # softmax sums at rows h*32

            def proj(nlo, nhi):
                for n in range(nlo, nhi):
                    nsl = bass.ts(n, NQ)
                    hts = []
                    for k in range(D // P):
                        htt = ht_pool.tile([P, NQ], BF16, name=f"ht{k}",
                                           tag="ht")
                        nc.sync.dma_start(htt[:], hT.ap()[bass.ts(k, P), nsl])
                        hts.append(htt)
                    for wn, bn, dest in (("wq", "bq", qT), ("wk", "bk", kT),
                                         ("wv", "bv", vT)):
                        ps = acc_ps.tile([P, NQ], F32, name=f"ps_{wn}",
                                         tag="acc")
                        for k in range(D // P):
                            nc.tensor.matmul(
                                ps[:], w_sbs[wn][:, bass.ts(k, P)], hts[k][:],
                                start=(k == 0), stop=(k == D // P - 1))
                        nc.vector.tensor_scalar_add(
                            dest[:, nsl], ps[:], b_sbs[bn][:])

            def attention(b):
                boff = b * S
                for h in range(2):
                    hsl = slice(h * HD, (h + 1) * HD)
                    va = va_pool.tile([P, KT * VA_W], BF16, name="va",
                                      tag="va")
                    nc.vector.memset(va[:], 1.0)
                    for kt in range(KT):
                        tp = acc_ps.tile([P, HD], BF16, name="tp", tag="acc")
                        nc.tensor.transpose(
                            tp[:], vT[hsl, boff + kt * P:boff + (kt + 1) * P],
                            ident[hsl, :])
                        nc.vector.tensor_copy(
                            va[:, kt * VA_W:kt * VA_W + HD], tp[:])
                    for c in range(2):   # 1024-wide query chunks
                        coff = boff + c * 1024
                        ctx = ctx_ps.tile([HD + 1, 1024], F32, name="ctx",
                                          tag="ctx")
                        for kt in range(KT):
                            sct = sc_ps.tile([P, 1024], F32, name="sct",
                                             tag="sct")
                            for i in range(2):
                                nc.tensor.matmul(
                                    sct[:, bass.ts(i, NQ)],
                                    kT[hsl, boff + kt * P:boff + (kt + 1) * P],
                                    qT[hsl, coff + i * NQ:coff + (i + 1) * NQ],
                                    start=True, stop=True)
                            pr = pr_pool.tile([P, 1024], BF16, name="pr",
                                              tag="pr")
                            nc.scalar.activation(
                                pr[:], sct[:],
                                mybir.ActivationFunctionType.Exp,
                                bias=mask_sb[:, b * KT + kt:b * KT + kt + 1],
                                scale=0.125)
                            for i in range(2):
                                nc.tensor.matmul(
                                    ctx[:, bass.ts(i, NQ)],
                                    va[:, kt * VA_W:(kt + 1) * VA_W],
                                    pr[:, bass.ts(i, NQ)],
                                    start=(kt == 0), stop=(kt == KT - 1))
                        # fast evacuation: two plain DVE copies release the
                        # ctx PSUM slot; reciprocal happens later off-PSUM
                        nc.vector.tensor_copy(
                            ctxraw[hsl, coff:coff + 1024], ctx[0:HD, :])
                        nc.vector.tensor_copy(
                            s_sb[h * 32:h * 32 + 1, coff:coff + 1024],
                            ctx[HD:HD + 1, :])

            def gather_oproj(b, clo, chi):
                """normalize + AllGather + output projection for GC-column
                chunks [clo, chi) of batch b."""
                boff = b * S
                for cg in range(clo, chi):
                    goff = boff + cg * GC
                    bcs = bc_pool.tile([P, GC], F32, name="bcs", tag="bcs")
                    for h in range(2):
                        rb = dram.tile([1, GC], F32, name=f"rb{b}{cg}{h}",
                                       tag=f"rb{b}{cg}{h}")
                        nc.sync.dma_start(
                            rb[:], s_sb[h * 32:h * 32 + 1, goff:goff + GC])
                        nc.sync.dma_start(
                            bcs[h * HD:(h + 1) * HD, :],
                            rb[:].to_broadcast((HD, GC)))
                    bcr = bc_pool.tile([P, GC], F32, name="bcr", tag="bcr")
                    nc.vector.reciprocal_approx_fast(bcr[:], bcs[:])
                    nc.vector.tensor_mul(
                        ctxn[:, goff:goff + GC], ctxraw[:, goff:goff + GC],
                        bcr[:])

                    cc_in = dram.tile([P, GC], BF16, name=f"cc_in{b}{cg}",
                                      tag=f"cc_in{b}{cg}")
                    cc_out = dram.tile([NCORES * P, GC], BF16,
                                       name=f"cc_out{b}{cg}",
                                       tag=f"cc_out{b}{cg}")
                    nc.sync.dma_start(cc_in[:], ctxn[:, goff:goff + GC])
                    nc.gpsimd.collective_compute(
                        "AllGather",
                        mybir.AluOpType.bypass,
                        replica_groups=[list(range(NCORES))],
                        ins=[cc_in[:].opt()],
                        outs=[cc_out[:].opt()],
                    )
                    po = acc_ps.tile([P, GC], F32, name="po", tag="acc")
                    for j in range(D // P):
                        gt = g_pool.tile([P, GC], BF16, name="gt", tag="gt")
                        nc.sync.dma_start(gt[:], cc_out[bass.ts(j, P), :])
                        nc.tensor.matmul(
                            po[:], w_sbs["wo"][:, bass.ts(j, P)], gt[:],
                            start=(j == 0), stop=(j == D // P - 1))
                    ot = ot_pool.tile([P, GC], F32, name="ot", tag="ot")
                    nc.vector.tensor_scalar_add(ot[:], po[:], b_sbs["bo"][:])
                    nc.sync.dma_start(outT.ap()[:, goff:goff + GC], ot[:])

            # emission order = scheduling priority: later phases act as
            # PE filler inside the ScalarE-bound attention windows
            proj(0, BS // NQ // 2)                  # A(b0)
            attention(0)                            # B(b0)
            proj(BS // NQ // 2, BS // NQ)           # A(b1)  (filler in B(b0))
            gather_oproj(0, 0, S // GC)             # C(b0)  (filler in B(b1))
            attention(1)                            # B(b1)
            gather_oproj(1, 0, S // GC)             # C(b1)  (tail)

    nc.compile()
    return nc


def _prep_inputs(hidden_state, attention_mask, Wq, bq, Wk, bk, Wv, bv, Wo, bo):
    h2 = np.ascontiguousarray(
        np.asarray(hidden_state, dtype=np.float32).reshape(BS, D).T
    ).astype(BF16_NP)
    maskT = np.ascontiguousarray(
        np.asarray(attention_mask, dtype=np.float32).reshape(B, S).T)
    in_maps = []
    for c in range(NCORES):
        sl = slice(c * P, (c + 1) * P)
        in_maps.append({
            "hT": h2,
            "wq": np.ascontiguousarray(np.asarray(Wq)[sl, :].T).astype(BF16_NP),
            "wk": np.ascontiguousarray(np.asarray(Wk)[sl, :].T).astype(BF16_NP),
            "wv": np.ascontiguousarray(np.asarray(Wv)[sl, :].T).astype(BF16_NP),
            "wo": np.ascontiguousarray(np.asarray(Wo)[sl, :].T).astype(BF16_NP),
            "bq": np.asarray(bq, dtype=np.float32)[sl].reshape(P, 1),
            "bk": np.asarray(bk, dtype=np.float32)[sl].reshape(P, 1),
            "bv": np.asarray(bv, dtype=np.float32)[sl].reshape(P, 1),
            "bo": np.asarray(bo, dtype=np.float32)[sl].reshape(P, 1),
            "maskT": maskT,
        })
    return in_maps


def kernel(**inputs) -> np.ndarray:
    if "nc" not in _CACHE:
        _CACHE["nc"] = _build()
    nc = _CACHE["nc"]
    in_maps = _prep_inputs(**inputs)
    res = bass_utils.run_bass_kernel_spmd(
        nc, in_maps, core_ids=list(range(NCORES)))
    outT = np.concatenate([res.results[c]["outT"] for c in range(NCORES)],
                          axis=0)            # [D, BS]
    return np.ascontiguousarray(outT.T).reshape(B, S, D).astype(np.float32)


# revision 16
# speedup vs baseline: 1.2122x; 1.0718x over previous
"""Multi-head attention (B=2, S=2048, D=1024, H=16) on 8 TRN2 NeuronCores.

Sharding: tensor-parallel on heads (2 heads = 128 channels per core).
Everything on-device runs in "transposed" layout [channel, B*S]:
  - host passes hiddenT [D, B*S] (bf16) replicated to all cores
  - per-core Q/K/V projections produce qT/kT/vT [128, B*S]
  - attention per (batch, head) in scoresT layout [key, query]:
      scoresT tile via matmul contracting the head dim, exp on ScalarE
      with the mask as per-partition bias and 1/sqrt(hd) as scale, the
      softmax denominator via an all-ones row appended to V (row 64 of
      the PV accumulator), normalization by DMA-broadcast reciprocal.
  - normalized ctxT (bf16) is AllGathered across cores in 512-column
    chunks; each core then computes a 128-row slice of outT = Wo @
    ctx.T per chunk and returns it.
Host concatenates the 8 slices and transposes back to [B, S, D].

Phase emission order is chosen so the TensorE always has ready "filler"
matmuls (projections for the other batch, output projection for the
previous batch) during the ScalarE-bound attention inner loop — this
both hides those phases and keeps the PE HAM clock-gate warm.
"""

import numpy as np
import ml_dtypes

import concourse.bass as bass
import concourse.mybir as mybir
import concourse.tile as tile
from concourse import bacc
from concourse import bass_utils
from concourse.masks import make_identity

F32 = mybir.dt.float32
BF16 = mybir.dt.bfloat16
BF16_NP = ml_dtypes.bfloat16

B, S, D, H = 2, 2048, 1024, 16
HD = D // H
BS = B * S            # 4096
P = 128               # partitions / channels per core
NCORES = 8
KT = S // P           # 16 key tiles per batch
NQ = 512              # matmul moving free dim
VA_W = HD + 1         # v_aug columns per key tile (64 v cols + ones col)
GC = 512              # gather / output-projection chunk (columns)

_CACHE = {}


def _build():
    nc = bacc.Bacc("TRN2", target_bir_lowering=False, debug=False,
                   num_devices=NCORES)

    hT = nc.dram_tensor("hT", [D, BS], BF16, kind="ExternalInput")
    wq = nc.dram_tensor("wq", [D, P], BF16, kind="ExternalInput")
    wk = nc.dram_tensor("wk", [D, P], BF16, kind="ExternalInput")
    wv = nc.dram_tensor("wv", [D, P], BF16, kind="ExternalInput")
    wo = nc.dram_tensor("wo", [D, P], BF16, kind="ExternalInput")
    bq = nc.dram_tensor("bq", [P, 1], F32, kind="ExternalInput")
    bk = nc.dram_tensor("bk", [P, 1], F32, kind="ExternalInput")
    bv = nc.dram_tensor("bv", [P, 1], F32, kind="ExternalInput")
    bo = nc.dram_tensor("bo", [P, 1], F32, kind="ExternalInput")
    maskT = nc.dram_tensor("maskT", [S, B], F32, kind="ExternalInput")
    outT = nc.dram_tensor("outT", [P, BS], F32, kind="ExternalOutput")

    with tile.TileContext(nc) as tc:
        with (
            tc.tile_pool(name="const", bufs=1) as const,
            tc.tile_pool(name="res", bufs=1) as res,
            tc.tile_pool(name="ht", bufs=16) as ht_pool,
            tc.tile_pool(name="va", bufs=2) as va_pool,
            tc.tile_pool(name="pr", bufs=3) as pr_pool,
            tc.tile_pool(name="bc", bufs=2) as bc_pool,
            tc.tile_pool(name="g", bufs=8) as g_pool,
            tc.tile_pool(name="ot", bufs=3) as ot_pool,
            tc.tile_pool(name="dram", bufs=1, space="DRAM") as dram,
            # PSUM: pj 1x[128,512](1 bank, projections+transposes) +
            #       po 1x[128,512](1, output projection) +
            #       sc 2x[128,1024](4) + ctx 1x[65,1024](2) = 8 banks
            tc.tile_pool(name="pj_ps", bufs=1, space="PSUM") as pj_ps,
            tc.tile_pool(name="po_ps", bufs=1, space="PSUM") as po_ps,
            tc.tile_pool(name="sc_ps", bufs=2, space="PSUM") as sc_ps,
            tc.tile_pool(name="ctx_ps", bufs=1, space="PSUM") as ctx_ps,
        ):
            # ---- constants / weights in SBUF ----
            w_sbs = {}
            for nm, w in (("wq", wq), ("wk", wk), ("wv", wv), ("wo", wo)):
                t = const.tile([P, D], BF16, name=f"{nm}_sb", tag=f"{nm}_sb")
                nc.sync.dma_start(
                    t[:].rearrange("p (j m) -> p j m", j=D // P),
                    w.ap().rearrange("(j p) m -> p j m", p=P))
                w_sbs[nm] = t
            b_sbs = {}
            for nm, bt in (("bq", bq), ("bk", bk), ("bv", bv), ("bo", bo)):
                t = const.tile([P, 1], F32, name=f"{nm}_sb", tag=f"{nm}_sb")
                nc.sync.dma_start(t[:], bt.ap())
                b_sbs[nm] = t
            mask_sb = const.tile([P, B * KT], F32)
            nc.sync.dma_start(
                mask_sb[:].rearrange("p (b t) -> p b t", b=B),
                maskT.ap().rearrange("(t p) b -> p b t", p=P))
            # identity replicated at base partitions 0 and 64 so transposes
            # of head-1 slices (base partition 64) have a matching-base rhs
            ident = const.tile([P, HD], BF16)
            make_identity(nc, ident[0:HD, :])
            nc.sync.dma_start(ident[HD:P, :], ident[0:HD, :])

            qT = res.tile([P, BS], BF16)
            kT = res.tile([P, BS], BF16)
            vT = res.tile([P, BS], BF16)
            ctxraw = res.tile([P, BS], F32)
            ctxn = res.tile([P, BS], BF16)
            s_sb = res.tile([64, BS], F32)   # softmax sums at rows h*32

            def proj(nlo, nhi):
                for n in range(nlo, nhi):
                    nsl = bass.ts(n, NQ)
                    hts = []
                    for k in range(D // P):
                        htt = ht_pool.tile([P, NQ], BF16, name=f"ht{k}",
                                           tag="ht")
                        nc.sync.dma_start(htt[:], hT.ap()[bass.ts(k, P), nsl])
                        hts.append(htt)
                    for wn, bn, dest in (("wq", "bq", qT), ("wk", "bk", kT),
                                         ("wv", "bv", vT)):
                        ps = pj_ps.tile([P, NQ], F32, name=f"ps_{wn}",
                                        tag="pj")
                        for k in range(D // P):
                            nc.tensor.matmul(
                                ps[:], w_sbs[wn][:, bass.ts(k, P)], hts[k][:],
                                start=(k == 0), stop=(k == D // P - 1))
                        nc.vector.tensor_scalar_add(
                            dest[:, nsl], ps[:], b_sbs[bn][:])

            def va_build(b):
                boff = b * S
                vas = []
                for h in range(2):
                    hsl = slice(h * HD, (h + 1) * HD)
                    va = va_pool.tile([P, KT * VA_W], BF16, name=f"va{b}{h}",
                                      tag=f"va{h}")
                    nc.vector.memset(va[:], 1.0)
                    for kt in range(KT):
                        tp = pj_ps.tile([P, HD], BF16, name="tp", tag="pj")
                        nc.tensor.transpose(
                            tp[:], vT[hsl, boff + kt * P:boff + (kt + 1) * P],
                            ident[hsl, :])
                        nc.vector.tensor_copy(
                            va[:, kt * VA_W:kt * VA_W + HD], tp[:])
                    vas.append(va)
                return vas

            def attn_chunk(b, h, c, va):
                """one head's attention for a 1024-wide query chunk"""
                boff = b * S
                hsl = slice(h * HD, (h + 1) * HD)
                coff = boff + c * 1024
                ctx = ctx_ps.tile([HD + 1, 1024], F32, name="ctx", tag="ctx")
                for kt in range(KT):
                    sct = sc_ps.tile([P, 1024], F32, name="sct", tag="sct")
                    for i in range(2):
                        nc.tensor.matmul(
                            sct[:, bass.ts(i, NQ)],
                            kT[hsl, boff + kt * P:boff + (kt + 1) * P],
                            qT[hsl, coff + i * NQ:coff + (i + 1) * NQ],
                            start=True, stop=True)
                    pr = pr_pool.tile([P, 1024], BF16, name="pr", tag="pr")
                    nc.scalar.activation(
                        pr[:], sct[:], mybir.ActivationFunctionType.Exp,
                        bias=mask_sb[:, b * KT + kt:b * KT + kt + 1],
                        scale=0.125)
                    for i in range(2):
                        nc.tensor.matmul(
                            ctx[:, bass.ts(i, NQ)],
                            va[:, kt * VA_W:(kt + 1) * VA_W],
                            pr[:, bass.ts(i, NQ)],
                            start=(kt == 0), stop=(kt == KT - 1))
                # fast evacuation: two plain DVE copies release the ctx PSUM
                # slot; reciprocal happens later off-PSUM
                nc.vector.tensor_copy(
                    ctxraw[hsl, coff:coff + 1024], ctx[0:HD, :])
                nc.vector.tensor_copy(
                    s_sb[h * 32:h * 32 + 1, coff:coff + 1024],
                    ctx[HD:HD + 1, :])

            def gather_oproj(b, clo, chi):
                """normalize + AllGather + output projection for GC-column
                chunks [clo, chi) of batch b."""
                boff = b * S
                for cg in range(clo, chi):
                    goff = boff + cg * GC
                    bcs = bc_pool.tile([P, GC], F32, name="bcs", tag="bcs")
                    for h in range(2):
                        rb = dram.tile([1, GC], F32, name=f"rb{b}{cg}{h}",
                                       tag=f"rb{b}{cg}{h}")
                        nc.sync.dma_start(
                            rb[:], s_sb[h * 32:h * 32 + 1, goff:goff + GC])
                        nc.sync.dma_start(
                            bcs[h * HD:(h + 1) * HD, :],
                            rb[:].to_broadcast((HD, GC)))
                    bcr = bc_pool.tile([P, GC], F32, name="bcr", tag="bcr")
                    nc.vector.reciprocal_approx_fast(bcr[:], bcs[:])
                    nc.vector.tensor_mul(
                        ctxn[:, goff:goff + GC], ctxraw[:, goff:goff + GC],
                        bcr[:])

                    cc_in = dram.tile([P, GC], BF16, name=f"cc_in{b}{cg}",
                                      tag=f"cc_in{b}{cg}")
                    cc_out = dram.tile([NCORES * P, GC], BF16,
                                       name=f"cc_out{b}{cg}",
                                       tag=f"cc_out{b}{cg}")
                    nc.sync.dma_start(cc_in[:], ctxn[:, goff:goff + GC])
                    nc.gpsimd.collective_compute(
                        "AllGather",
                        mybir.AluOpType.bypass,
                        replica_groups=[list(range(NCORES))],
                        ins=[cc_in[:].opt()],
                        outs=[cc_out[:].opt()],
                    )
                    po = po_ps.tile([P, GC], F32, name="po", tag="po")
                    for j in range(D // P):
                        gt = g_pool.tile([P, GC], BF16, name="gt", tag="gt")
                        nc.sync.dma_start(gt[:], cc_out[bass.ts(j, P), :])
                        nc.tensor.matmul(
                            po[:], w_sbs["wo"][:, bass.ts(j, P)], gt[:],
                            start=(j == 0), stop=(j == D // P - 1))
                    ot = ot_pool.tile([P, GC], F32, name="ot", tag="ot")
                    nc.vector.tensor_scalar_add(ot[:], po[:], b_sbs["bo"][:])
                    nc.sync.dma_start(outT.ap()[:, goff:goff + GC], ot[:])

            # emission order = scheduling priority: later phases act as
            # PE filler inside the ScalarE-bound attention windows; gathers
            # are launched per 1024-column chunk as soon as both heads of
            # that chunk are done, so the collective stream overlaps the
            # rest of the attention.
            proj(0, BS // NQ // 2)                  # A(b0)
            va0 = va_build(0)
            attn_chunk(0, 0, 0, va0[0])
            attn_chunk(0, 1, 0, va0[1])
            proj(BS // NQ // 2, BS // NQ)           # A(b1)  (filler)
            va1 = va_build(1)
            gather_oproj(0, 0, 2)                   # C(b0) first half
            attn_chunk(0, 0, 1, va0[0])
            attn_chunk(0, 1, 1, va0[1])
            gather_oproj(0, 2, 4)                   # C(b0) second half
            attn_chunk(1, 0, 0, va1[0])
            attn_chunk(1, 1, 0, va1[1])
            gather_oproj(1, 0, 2)                   # C(b1) first half
            attn_chunk(1, 0, 1, va1[0])
            attn_chunk(1, 1, 1, va1[1])
            gather_oproj(1, 2, 4)                   # C(b1) tail

    nc.compile()
    return nc


def _prep_inputs(hidden_state, attention_mask, Wq, bq, Wk, bk, Wv, bv, Wo, bo):
    h2 = np.ascontiguousarray(
        np.asarray(hidden_state, dtype=np.float32).reshape(BS, D).T
    ).astype(BF16_NP)
    maskT = np.ascontiguousarray(
        np.asarray(attention_mask, dtype=np.float32).reshape(B, S).T)
    in_maps = []
    for c in range(NCORES):
        sl = slice(c * P, (c + 1) * P)
        in_maps.append({
            "hT": h2,
            "wq": np.ascontiguousarray(np.asarray(Wq)[sl, :].T).astype(BF16_NP),
            "wk": np.ascontiguousarray(np.asarray(Wk)[sl, :].T).astype(BF16_NP),
            "wv": np.ascontiguousarray(np.asarray(Wv)[sl, :].T).astype(BF16_NP),
            "wo": np.ascontiguousarray(np.asarray(Wo)[sl, :].T).astype(BF16_NP),
            "bq": np.asarray(bq, dtype=np.float32)[sl].reshape(P, 1),
            "bk": np.asarray(bk, dtype=np.float32)[sl].reshape(P, 1),
            "bv": np.asarray(bv, dtype=np.float32)[sl].reshape(P, 1),
            "bo": np.asarray(bo, dtype=np.float32)[sl].reshape(P, 1),
            "maskT": maskT,
        })
    return in_maps


def kernel(**inputs) -> np.ndarray:
    if "nc" not in _CACHE:
        _CACHE["nc"] = _build()
    nc = _CACHE["nc"]
    in_maps = _prep_inputs(**inputs)
    res = bass_utils.run_bass_kernel_spmd(
        nc, in_maps, core_ids=list(range(NCORES)))
    outT = np.concatenate([res.results[c]["outT"] for c in range(NCORES)],
                          axis=0)            # [D, BS]
    return np.ascontiguousarray(outT.T).reshape(B, S, D).astype(np.float32)


# revision 17
# speedup vs baseline: 1.2598x; 1.0393x over previous
"""Multi-head attention (B=2, S=2048, D=1024, H=16) on 8 TRN2 NeuronCores.

Sharding: tensor-parallel on heads (2 heads = 128 channels per core).
Everything on-device runs in "transposed" layout [channel, B*S]:
  - host passes hiddenT [D, B*S] (bf16) replicated to all cores
  - per-core Q/K/V projections produce qT/kT/vT [128, B*S]
  - attention per (batch, head) in scoresT layout [key, query]:
      scoresT tile via matmul contracting the head dim, exp on ScalarE
      with the mask as per-partition bias and 1/sqrt(hd) as scale, the
      softmax denominator via an all-ones row appended to V (row 64 of
      the PV accumulator), normalization by DMA-broadcast reciprocal.
  - normalized ctxT (bf16) is AllGathered across cores in 512-column
    chunks; each core then computes a 128-row slice of outT = Wo @
    ctx.T per chunk and returns it.
Host concatenates the 8 slices and transposes back to [B, S, D].

Phase emission order is chosen so the TensorE always has ready "filler"
matmuls (projections for the other batch, output projection for the
previous batch) during the ScalarE-bound attention inner loop — this
both hides those phases and keeps the PE HAM clock-gate warm.
"""

import numpy as np
import ml_dtypes

import concourse.bass as bass
import concourse.mybir as mybir
import concourse.tile as tile
from concourse import bacc
from concourse import bass_utils
from concourse.masks import make_identity

F32 = mybir.dt.float32
BF16 = mybir.dt.bfloat16
BF16_NP = ml_dtypes.bfloat16

B, S, D, H = 2, 2048, 1024, 16
HD = D // H
BS = B * S            # 4096
P = 128               # partitions / channels per core
NCORES = 8
KT = S // P           # 16 key tiles per batch
NQ = 512              # matmul moving free dim
VA_W = HD + 1         # v_aug columns per key tile (64 v cols + ones col)
GC = 512              # gather / output-projection chunk (columns)

_CACHE = {}


def _build():
    nc = bacc.Bacc("TRN2", target_bir_lowering=False, debug=False,
                   num_devices=NCORES)

    hT = nc.dram_tensor("hT", [D, BS], BF16, kind="ExternalInput")
    wq = nc.dram_tensor("wq", [D, P], BF16, kind="ExternalInput")
    wk = nc.dram_tensor("wk", [D, P], BF16, kind="ExternalInput")
    wv = nc.dram_tensor("wv", [D, P], BF16, kind="ExternalInput")
    wo = nc.dram_tensor("wo", [D, P], BF16, kind="ExternalInput")
    bq = nc.dram_tensor("bq", [P, 1], F32, kind="ExternalInput")
    bk = nc.dram_tensor("bk", [P, 1], F32, kind="ExternalInput")
    bv = nc.dram_tensor("bv", [P, 1], F32, kind="ExternalInput")
    bo = nc.dram_tensor("bo", [P, 1], F32, kind="ExternalInput")
    maskT = nc.dram_tensor("maskT", [S, B], F32, kind="ExternalInput")
    outT = nc.dram_tensor("outT", [P, BS], F32, kind="ExternalOutput")

    with tile.TileContext(nc) as tc:
        with (
            tc.tile_pool(name="const", bufs=1) as const,
            tc.tile_pool(name="res", bufs=1) as res,
            tc.tile_pool(name="ht", bufs=16) as ht_pool,
            tc.tile_pool(name="va", bufs=2) as va_pool,
            tc.tile_pool(name="pr", bufs=3) as pr_pool,
            tc.tile_pool(name="bc", bufs=2) as bc_pool,
            tc.tile_pool(name="g", bufs=8) as g_pool,
            tc.tile_pool(name="ot", bufs=3) as ot_pool,
            tc.tile_pool(name="dram", bufs=1, space="DRAM") as dram,
            # PSUM: pj 1x[128,512](1 bank, projections+transposes) +
            #       po 1x[128,512](1, output projection) +
            #       sc 2x[128,1024](4) + ctx 1x[65,1024](2) = 8 banks
            tc.tile_pool(name="pj_ps", bufs=1, space="PSUM") as pj_ps,
            tc.tile_pool(name="po_ps", bufs=1, space="PSUM") as po_ps,
            tc.tile_pool(name="sc_ps", bufs=2, space="PSUM") as sc_ps,
            tc.tile_pool(name="ctx_ps", bufs=1, space="PSUM") as ctx_ps,
        ):
            # ---- constants / weights in SBUF ----
            w_sbs = {}
            for nm, w in (("wq", wq), ("wk", wk), ("wv", wv), ("wo", wo)):
                t = const.tile([P, D], BF16, name=f"{nm}_sb", tag=f"{nm}_sb")
                nc.sync.dma_start(
                    t[:].rearrange("p (j m) -> p j m", j=D // P),
                    w.ap().rearrange("(j p) m -> p j m", p=P))
                w_sbs[nm] = t
            b_sbs = {}
            for nm, bt in (("bq", bq), ("bk", bk), ("bv", bv), ("bo", bo)):
                t = const.tile([P, 1], F32, name=f"{nm}_sb", tag=f"{nm}_sb")
                nc.sync.dma_start(t[:], bt.ap())
                b_sbs[nm] = t
            mask_sb = const.tile([P, B * KT], F32)
            nc.sync.dma_start(
                mask_sb[:].rearrange("p (b t) -> p b t", b=B),
                maskT.ap().rearrange("(t p) b -> p b t", p=P))
            # identity replicated at base partitions 0 and 64 so transposes
            # of head-1 slices (base partition 64) have a matching-base rhs
            ident = const.tile([P, HD], BF16)
            make_identity(nc, ident[0:HD, :])
            nc.sync.dma_start(ident[HD:P, :], ident[0:HD, :])

            qT = res.tile([P, BS], BF16)
            kT = res.tile([P, BS], BF16)
            vT = res.tile([P, BS], BF16)
            ctxraw = res.tile([P, BS], F32)
            ctxn = res.tile([P, BS], BF16)
            s_sb = res.tile([64, BS], F32)   # softmax sums at rows h*32

            VA = {}

            def batch_inputs_steps(b):
                """Projections + v_aug build for batch b as a generator of
                small emission steps (used as PE filler inside attention)."""
                vas = []
                for h in range(2):
                    va = va_pool.tile([P, KT * VA_W], BF16, name=f"va{b}{h}",
                                      tag=f"va{h}")
                    nc.vector.memset(va[:], 1.0)
                    vas.append(va)
                VA[b] = vas
                boff = b * S
                for n in range(b * 4, (b + 1) * 4):
                    nsl = bass.ts(n, NQ)
                    hts = []
                    for k in range(D // P):
                        htt = ht_pool.tile([P, NQ], BF16, name=f"ht{k}",
                                           tag="ht")
                        nc.sync.dma_start(htt[:], hT.ap()[bass.ts(k, P), nsl])
                        hts.append(htt)
                    yield
                    for wn, bn, dest in (("wq", "bq", qT), ("wk", "bk", kT),
                                         ("wv", "bv", vT)):
                        ps = pj_ps.tile([P, NQ], F32, name=f"ps_{wn}",
                                        tag="pj")
                        for k in range(D // P):
                            nc.tensor.matmul(
                                ps[:], w_sbs[wn][:, bass.ts(k, P)], hts[k][:],
                                start=(k == 0), stop=(k == D // P - 1))
                            if k % 4 == 3:
                                yield
                        nc.vector.tensor_scalar_add(
                            dest[:, nsl], ps[:], b_sbs[bn][:])
                        yield
                    # vT for this 512-col chunk is done -> its 4 key tiles
                    # can be transposed into v_aug
                    nlocal = n - b * 4
                    for kt in range(nlocal * 4, nlocal * 4 + 4):
                        for h in range(2):
                            hsl = slice(h * HD, (h + 1) * HD)
                            tp = pj_ps.tile([P, HD], BF16, name="tp",
                                            tag="pj")
                            nc.tensor.transpose(
                                tp[:],
                                vT[hsl, boff + kt * P:boff + (kt + 1) * P],
                                ident[hsl, :])
                            nc.vector.tensor_copy(
                                vas[h][:, kt * VA_W:kt * VA_W + HD], tp[:])
                        yield

            def attn_chunk(b, h, c, filler):
                """one head's attention for a 1024-wide query chunk; pops one
                filler step per key tile to keep the PE stream dense"""
                va = VA[b][h]
                boff = b * S
                hsl = slice(h * HD, (h + 1) * HD)
                coff = boff + c * 1024
                ctx = ctx_ps.tile([HD + 1, 1024], F32, name="ctx", tag="ctx")
                for kt in range(KT):
                    sct = sc_ps.tile([P, 1024], F32, name="sct", tag="sct")
                    for i in range(2):
                        nc.tensor.matmul(
                            sct[:, bass.ts(i, NQ)],
                            kT[hsl, boff + kt * P:boff + (kt + 1) * P],
                            qT[hsl, coff + i * NQ:coff + (i + 1) * NQ],
                            start=True, stop=True)
                    pr = pr_pool.tile([P, 1024], BF16, name="pr", tag="pr")
                    nc.scalar.activation(
                        pr[:], sct[:], mybir.ActivationFunctionType.Exp,
                        bias=mask_sb[:, b * KT + kt:b * KT + kt + 1],
                        scale=0.125)
                    for i in range(2):
                        nc.tensor.matmul(
                            ctx[:, bass.ts(i, NQ)],
                            va[:, kt * VA_W:(kt + 1) * VA_W],
                            pr[:, bass.ts(i, NQ)],
                            start=(kt == 0), stop=(kt == KT - 1))
                    if filler is not None:
                        next(filler, None)
                # fast evacuation: two plain DVE copies release the ctx PSUM
                # slot; reciprocal happens later off-PSUM
                nc.vector.tensor_copy(
                    ctxraw[hsl, coff:coff + 1024], ctx[0:HD, :])
                nc.vector.tensor_copy(
                    s_sb[h * 32:h * 32 + 1, coff:coff + 1024],
                    ctx[HD:HD + 1, :])

            def gather_norm(b, clo, chi):
                """normalize ctxT and trigger the AllGather for GC-column
                chunks [clo, chi) of batch b (no PSUM work, no long waits)"""
                boff = b * S
                for cg in range(clo, chi):
                    goff = boff + cg * GC
                    bcs = bc_pool.tile([P, GC], F32, name="bcs", tag="bcs")
                    for h in range(2):
                        rb = dram.tile([1, GC], F32, name=f"rb{b}{cg}{h}",
                                       tag=f"rb{b}{cg}{h}")
                        nc.sync.dma_start(
                            rb[:], s_sb[h * 32:h * 32 + 1, goff:goff + GC])
                        nc.sync.dma_start(
                            bcs[h * HD:(h + 1) * HD, :],
                            rb[:].to_broadcast((HD, GC)))
                    bcr = bc_pool.tile([P, GC], F32, name="bcr", tag="bcr")
                    nc.vector.reciprocal_approx_fast(bcr[:], bcs[:])
                    nc.vector.tensor_mul(
                        ctxn[:, goff:goff + GC], ctxraw[:, goff:goff + GC],
                        bcr[:])
                    cc_in = dram.tile([P, GC], BF16, name=f"cc_in{b}{cg}",
                                      tag=f"cc_in{b}{cg}")
                    cc_out = dram.tile([NCORES * P, GC], BF16,
                                       name=f"cc_out{b}{cg}",
                                       tag=f"cc_out{b}{cg}")
                    CC_OUT[(b, cg)] = cc_out
                    nc.sync.dma_start(cc_in[:], ctxn[:, goff:goff + GC])
                    nc.gpsimd.collective_compute(
                        "AllGather",
                        mybir.AluOpType.bypass,
                        replica_groups=[list(range(NCORES))],
                        ins=[cc_in[:].opt()],
                        outs=[cc_out[:].opt()],
                    )

            CC_OUT = {}

            def oproj_steps(b):
                """output projection for batch b; emitted only at stream
                positions where its AllGathers are already complete"""
                boff = b * S
                for cg in range(S // GC):
                    goff = boff + cg * GC
                    cc_out = CC_OUT[(b, cg)]
                    po = po_ps.tile([P, GC], F32, name="po", tag="po")
                    for j in range(D // P):
                        gt = g_pool.tile([P, GC], BF16, name="gt", tag="gt")
                        nc.sync.dma_start(gt[:], cc_out[bass.ts(j, P), :])
                        nc.tensor.matmul(
                            po[:], w_sbs["wo"][:, bass.ts(j, P)], gt[:],
                            start=(j == 0), stop=(j == D // P - 1))
                        if j % 4 == 3:
                            yield
                    ot = ot_pool.tile([P, GC], F32, name="ot", tag="ot")
                    nc.vector.tensor_scalar_add(ot[:], po[:], b_sbs["bo"][:])
                    nc.sync.dma_start(outT.ap()[:, goff:goff + GC], ot[:])
                    yield

            def drain(g):
                for _ in g:
                    pass

            # software pipeline: engines run their streams in-order, so
            # anything that waits on a slow dependency must sit at a stream
            # position where that dependency is already resolved.
            drain(batch_inputs_steps(0))        # A(b0) up front
            fillerA = batch_inputs_steps(1)     # A(b1): filler inside B(b0)
            attn_chunk(0, 0, 0, fillerA)
            attn_chunk(0, 1, 0, fillerA)
            gather_norm(0, 0, 2)
            attn_chunk(0, 0, 1, fillerA)
            attn_chunk(0, 1, 1, fillerA)
            drain(fillerA)
            gather_norm(0, 2, 4)
            fillerB = oproj_steps(0)            # C(b0): filler inside B(b1)
            attn_chunk(1, 0, 0, fillerB)
            attn_chunk(1, 1, 0, fillerB)
            gather_norm(1, 0, 2)
            attn_chunk(1, 0, 1, fillerB)
            attn_chunk(1, 1, 1, fillerB)
            drain(fillerB)
            gather_norm(1, 2, 4)
            drain(oproj_steps(1))               # C(b1) tail

    nc.compile()
    return nc


def _prep_inputs(hidden_state, attention_mask, Wq, bq, Wk, bk, Wv, bv, Wo, bo):
    h2 = np.ascontiguousarray(
        np.asarray(hidden_state, dtype=np.float32).reshape(BS, D).T
    ).astype(BF16_NP)
    maskT = np.ascontiguousarray(
        np.asarray(attention_mask, dtype=np.float32).reshape(B, S).T)
    in_maps = []
    for c in range(NCORES):
        sl = slice(c * P, (c + 1) * P)
        in_maps.append({
            "hT": h2,
            "wq": np.ascontiguousarray(np.asarray(Wq)[sl, :].T).astype(BF16_NP),
            "wk": np.ascontiguousarray(np.asarray(Wk)[sl, :].T).astype(BF16_NP),
            "wv": np.ascontiguousarray(np.asarray(Wv)[sl, :].T).astype(BF16_NP),
            "wo": np.ascontiguousarray(np.asarray(Wo)[sl, :].T).astype(BF16_NP),
            "bq": np.asarray(bq, dtype=np.float32)[sl].reshape(P, 1),
            "bk": np.asarray(bk, dtype=np.float32)[sl].reshape(P, 1),
            "bv": np.asarray(bv, dtype=np.float32)[sl].reshape(P, 1),
            "bo": np.asarray(bo, dtype=np.float32)[sl].reshape(P, 1),
            "maskT": maskT,
        })
    return in_maps


def kernel(**inputs) -> np.ndarray:
    if "nc" not in _CACHE:
        _CACHE["nc"] = _build()
    nc = _CACHE["nc"]
    in_maps = _prep_inputs(**inputs)
    res = bass_utils.run_bass_kernel_spmd(
        nc, in_maps, core_ids=list(range(NCORES)))
    outT = np.concatenate([res.results[c]["outT"] for c in range(NCORES)],
                          axis=0)            # [D, BS]
    return np.ascontiguousarray(outT.T).reshape(B, S, D).astype(np.float32)
